# revision 7
# baseline (speedup 1.0000x reference)
"""Conv4d (3,3,3,3) kernel for Trainium2, 8 NeuronCores.

Problem: x (2,24,16,16,48,48) * weight (48,24,3,3,3,3) + bias3d.sum(0)
      -> out (2,48,14,14,46,46), stride 1, no padding.

Strategy
--------
Sharding: 8 cores = (batch 2) x (ol-block 2) x (od-block 2). Each core owns a
7x7 block of (ol, od) output planes (49 planes) and receives the overlapping
input slab x[b, :, 7*lb : 7*lb+9, 7*db : 7*db+9, :, :] flattened to
[24, 9, 9, 2306] (HW plane padded 2304 -> 2306 with zeros so shifted matmul
reads never leave the tile).

Per output plane (ol, od): implicit GEMM. Contraction rows = (lo, do, ci)
= 9*24 = 216, split into two stationary-weight tiles of K=120 and K=96(+1).
For each of the 9 (ho, wo) kernel offsets the moving operand is the same
SBUF-resident input tile read at free offset (oh+ho)*48 + wo; all 18 matmuls
accumulate into one PSUM tile holding a chunk of output rows at padded
W-pitch 48 (junk columns ow=46,47 are dropped on the host). The bias is an
extra contraction row: weight row 96 of the second tile is sum(bias3d) for
offset (0,0) only, matched against an all-ones input row.

dtype float32r: fp32 operands, reduced-precision PE mode, 1 cycle/row for
free dim >= 256 (4x faster than plain fp32 matmul).

Column split: successive output chunks alternate between PE column halves
(PSUM partitions 0:48 and 64:112) so two matmul streams occupy disjoint
32-column groups of the systolic array and run concurrently.
"""

import sys

if "/opt/trn_rl_repo" not in sys.path:
    sys.path.insert(0, "/opt/trn_rl_repo")

from itertools import product

import numpy as np

from concourse import bass, bacc, tile
from concourse.bass_utils import run_bass_kernel_spmd

mybir = bass.mybir

B, CI, CO = 2, 24, 48
L, D, H, W = 16, 16, 48, 48
OL, OD, OH, OW = 14, 14, 46, 46
F = H * W  # 2304, one (h, w) plane per partition row
FPAD = F + 2  # shifted reads go up to (45+2)*48 + 2 + ... = 2306
N_TASKS = 49  # 7x7 (ol, od) planes per core

# output row chunking: 8+8+8+8+8+6 rows, N = rows*48 (all >= 256 for full
# fp32r rate, all <= 512 = PSUM bank / fp32 moving-operand limit)
CHUNK_ROWS = (8, 8, 8, 8, 8, 6)
CHUNK_OH0 = (0, 8, 16, 24, 32, 40)

import os

_DTYPE_NAMES = {
    "f32r": mybir.dt.float32r,
    "bf16": mybir.dt.bfloat16,
    "f16": mybir.dt.float16,
    "f32": mybir.dt.float32,
}
DTYPE = _DTYPE_NAMES[os.environ.get("CONV_DTYPE", "f32r")]
COLSPLIT = os.environ.get("CONV_COLSPLIT", "1") == "1"
X_BUFS = 3
PS_BUFS = 6
O_BUFS = 6


def _np_dtype():
    return mybir.dt.np(DTYPE)


def build_program(n_tasks: int = N_TASKS):
    nc = bacc.Bacc()
    f32 = mybir.dt.float32

    # xs: per-task packed contraction rows. Row r = (lo*3+do)*24+ci holds the
    # input plane x[b, ci, ol+lo, od+do, :, :]; row 216 is all-ones (bias).
    xs_d = nc.dram_tensor("xs", [n_tasks, 217, FPAD], DTYPE, kind="ExternalInput")
    w1_d = nc.dram_tensor("w1", [120, 9, CO], DTYPE, kind="ExternalInput")
    w2_d = nc.dram_tensor("w2", [97, 9, CO], DTYPE, kind="ExternalInput")
    out_d = nc.dram_tensor("out", [n_tasks, CO, OH, 48], f32, kind="ExternalOutput")

    with tile.TileContext(nc) as tc:
        with (
            tc.tile_pool(name="wpool", bufs=1) as wpool,
            tc.tile_pool(name="xpool", bufs=X_BUFS) as xpool,
            tc.tile_pool(name="opool", bufs=O_BUFS) as opool,
            tc.tile_pool(name="pspool", bufs=PS_BUFS, space="PSUM") as pspool,
        ):
            w1s = wpool.tile([120, 9, CO], DTYPE)
            w2s = wpool.tile([97, 9, CO], DTYPE)
            nc.sync.dma_start(out=w1s[:], in_=w1_d[:])
            nc.sync.dma_start(out=w2s[:], in_=w2_d[:])

            for t in range(n_tasks):
                k1 = xpool.tile([120, FPAD], DTYPE, tag="k1")
                k2 = xpool.tile([97, FPAD], DTYPE, tag="k2")
                nc.sync.dma_start(out=k1[:], in_=xs_d[t, 0:120, :])
                nc.sync.dma_start(out=k2[:], in_=xs_d[t, 120:217, :])

                # chunk pairs share PE: even chunk -> columns 0:48,
                # odd chunk -> columns 64:112 (disjoint column groups).
                for ci0 in range(0, len(CHUNK_ROWS), 2):
                    pair = [ci0] + ([ci0 + 1] if ci0 + 1 < len(CHUNK_ROWS) else [])
                    ps_l, o_l = [], []
                    for c in pair:
                        rows = CHUNK_ROWS[c]
                        if COLSPLIT and (c % 2 == 1):
                            ps_full = pspool.tile([112, 8, 48], mybir.dt.float32, tag="ps")
                            o_full = opool.tile([112, 8, 48], mybir.dt.float32, tag="o")
                            ps_l.append(ps_full[64:112, :rows, :])
                            o_l.append(o_full[64:112, :rows, :])
                        else:
                            ps_full = pspool.tile([112, 8, 48], mybir.dt.float32, tag="ps")
                            o_full = opool.tile([112, 8, 48], mybir.dt.float32, tag="o")
                            ps_l.append(ps_full[0:CO, :rows, :])
                            o_l.append(o_full[0:CO, :rows, :])
                    # interleave the two chunks' matmuls so the PE can run
                    # both column halves concurrently
                    for idx, (ho, wo) in enumerate(product(range(3), range(3))):
                        for j, c in enumerate(pair):
                            rows = CHUNK_ROWS[c]
                            n = rows * 48
                            off = (CHUNK_OH0[c] + ho) * 48 + wo
                            nc.tensor.matmul(
                                ps_l[j],
                                lhsT=w1s[:, idx, :],
                                rhs=k1[:, off : off + n],
                                start=(idx == 0),
                                stop=False,
                            )
                            nc.tensor.matmul(
                                ps_l[j],
                                lhsT=w2s[:, idx, :],
                                rhs=k2[:, off : off + n],
                                start=False,
                                stop=(idx == 8),
                            )
                    for j, c in enumerate(pair):
                        rows = CHUNK_ROWS[c]
                        nc.vector.tensor_copy(out=o_l[j], in_=ps_l[j])
                        nc.gpsimd.dma_start(
                            out=out_d[t, :, CHUNK_OH0[c] : CHUNK_OH0[c] + rows, :],
                            in_=o_l[j],
                        )
    nc.finalize()
    return nc


def make_in_maps(x, weight, bias3d, n_tasks: int = N_TASKS):
    """Host-side shard + repack into the per-task packed-row layout."""
    npdt = _np_dtype()
    x = np.asarray(x, np.float32)
    weight = np.asarray(weight, np.float32)
    bias3d = np.asarray(bias3d, np.float32)

    # W[(lo*3+do)*24+ci, ho*3+wo, co] = weight[co, ci, lo, do, ho, wo]
    Wr = np.ascontiguousarray(np.transpose(weight, (2, 3, 1, 4, 5, 0))).reshape(
        216, 9, CO
    )
    w1 = np.ascontiguousarray(Wr[:120]).astype(npdt)
    w2 = np.zeros((97, 9, CO), np.float32)
    w2[:96] = Wr[120:]
    w2[96, 0, :] = bias3d.sum(axis=0)
    w2 = w2.astype(npdt)

    in_maps = []
    for c in range(8):
        b, lb, db = c // 4, (c // 2) % 2, c % 2
        slab = np.ascontiguousarray(
            x[b, :, 7 * lb : 7 * lb + 9, 7 * db : 7 * db + 9]
        )  # (24, 9, 9, 48, 48) -> strides for (ci, l, d, f)
        slab = slab.reshape(CI, 9, 9, F)
        s_ci, s_l, s_d, s_f = slab.strides
        # V[l0, d0, lo, do, ci, f] = slab[ci, l0+lo, d0+do, f]
        V = np.lib.stride_tricks.as_strided(
            slab,
            shape=(7, 7, 3, 3, CI, F),
            strides=(s_l, s_d, s_l, s_d, s_ci, s_f),
        )
        xs = np.zeros((N_TASKS, 217, FPAD), np.float32)
        xs[:, :216, :F] = V.reshape(N_TASKS, 216, F)
        xs[:, 216, :] = 1.0
        in_maps.append({"xs": xs[:n_tasks].astype(npdt), "w1": w1, "w2": w2})
    return in_maps


def assemble_output(results):
    out = np.empty((B, CO, OL, OD, OH, OW), np.float32)
    for c in range(8):
        b, lb, db = c // 4, (c // 2) % 2, c % 2
        r = results[c]["out"]  # (49, 48, 46, 48)
        r = r.reshape(7, 7, CO, OH, 48)[:, :, :, :, :OW]
        out[b, :, 7 * lb : 7 * lb + 7, 7 * db : 7 * db + 7] = r.transpose(2, 0, 1, 3, 4)
    return out


_NC_CACHE = {}


def _get_program():
    if "nc" not in _NC_CACHE:
        _NC_CACHE["nc"] = build_program()
    return _NC_CACHE["nc"]


def kernel(x, weight, bias3d):
    nc = _get_program()
    in_maps = make_in_maps(x, weight, bias3d)
    res = run_bass_kernel_spmd(nc, in_maps, list(range(8))).results
    return assemble_output(res)


# revision 22
# speedup vs baseline: 62179.5626x; 62179.5626x over previous
"""Conv4d (3,3,3,3) kernel for Trainium2, 8 NeuronCores.

Problem: x (2,24,16,16,48,48) * weight (48,24,3,3,3,3) + bias3d.sum(0)
      -> out (2,48,14,14,46,46), stride 1, no padding.

Strategy
--------
Sharding: 8 cores = (batch 2) x (ol-block 2) x (od-block 2). Each core owns a
7x7 block of (ol, od) output planes (49 tasks).

Per task: implicit GEMM. Contraction rows = (lo, do, ci) = 216 (+1 bias row),
packed on the host into xs[t, 217, 48, 48] where row r = (lo*3+do)*24+ci is
the input plane x[b, ci, ol+lo, od+do, :, :]; row 216 is all-ones. For each
of the 9 (ho, wo) kernel offsets the moving operand is the same SBUF-resident
tile sliced [k, oh0+ho : oh0+ho+rows, wo : wo+46]; all offsets accumulate
into one PSUM tile of output rows [48, rows, 46]. Bias is weight row 216
(offset (0,0) only) against the ones row.

dtype fp16 (default): 1 cycle/row on the PE, ~3e-4 scale-relative error
after fp32 PSUM accumulation over 1944 terms (weights/activations are well
inside fp16 range). CONV_DTYPE=f32r gives full fp32 operand storage at the
same matmul rate (~1.4e-4) at 2x the DMA bytes.

Measured on HW (repeat-loop delta, 8 cores): ~1.0 ms per kernel execution,
~90 matmuls x ~460-element streams per task, PE-serial bound. Column-half
tile_position concurrency and weight-stationary reorderings were measured
and gave no speedup in the full kernel (see CONV_COLSPLIT knob); input-DMA
SBUF writes account for ~16% of the span.
"""

import os
import sys

if "/opt/trn_rl_repo" not in sys.path:
    sys.path.insert(0, "/opt/trn_rl_repo")

from contextlib import nullcontext

import numpy as np

from concourse import bacc, bass, tile
from concourse.bass_utils import run_bass_kernel_spmd

mybir = bass.mybir

B, CI, CO = 2, 24, 48
L, D, H, W = 16, 16, 48, 48
OL, OD, OH, OW = 14, 14, 46, 46
N_TASKS = 49  # 7x7 (ol, od) planes per core
KROWS = 217  # (lo,do,ci) contraction rows + ones row
KSPLIT = 128  # k1 = rows 0:128, k2 = rows 128:217

# output row chunks: N = rows*46 <= 512 (PSUM bank), >= 256 (fp32r full rate)
if os.environ.get("CONV_CHUNK8", "0") == "1":
    CHUNK_ROWS = (8, 8, 8, 8, 8, 6)
    CHUNK_OH0 = (0, 8, 16, 24, 32, 40)
else:
    CHUNK_ROWS = (10, 10, 10, 10, 6)
    CHUNK_OH0 = (0, 10, 20, 30, 40)

ACTCOPY = os.environ.get("CONV_ACTCOPY", "0") == "1"
_DTYPE_NAMES = {
    "f32r": mybir.dt.float32r,
    "bf16": mybir.dt.bfloat16,
    "f16": mybir.dt.float16,
    "f32": mybir.dt.float32,
}
DTYPE = _DTYPE_NAMES[os.environ.get("CONV_DTYPE", "f16")]
COLSPLIT = os.environ.get("CONV_COLSPLIT", "0") == "1"
# PITCH=46: exact-width 3D rhs/psum APs; PITCH=48: padded-width contiguous 1D
# rhs (junk output columns ow=46,47 dropped on host)
PITCH = int(os.environ.get("CONV_PITCH", "46"))
X_BUFS = int(os.environ.get("CONV_XBUFS", "3"))
PS_BUFS = int(os.environ.get("CONV_PSBUFS", "8"))
O_BUFS = int(os.environ.get("CONV_OBUFS", "6"))
# bench-only bisection knobs (break correctness!)
NOEPI = os.environ.get("CONV_NOEPI", "0") == "1"  # skip copy + dma-out
NOLOAD = os.environ.get("CONV_NOLOAD", "0") == "1"  # skip k-tile loads


def _np_dtype():
    return mybir.dt.np(DTYPE)


def build_program(n_tasks: int = N_TASKS, repeat: int = 1):
    nc = bacc.Bacc()
    f32 = mybir.dt.float32
    k2rows = KROWS - KSPLIT  # 89
    p48 = PITCH == 48
    FPAD = 2306  # flat plane + 2 pad elems for the largest shifted read

    if p48:
        xs_d = nc.dram_tensor("xs", [n_tasks, KROWS, FPAD], DTYPE, kind="ExternalInput")
        out_d = nc.dram_tensor("out", [n_tasks, CO, OH, 48], f32, kind="ExternalOutput")
    else:
        xs_d = nc.dram_tensor("xs", [n_tasks, KROWS, H, W], DTYPE, kind="ExternalInput")
        out_d = nc.dram_tensor("out", [n_tasks, CO, OH, OW], f32, kind="ExternalOutput")
    w1_d = nc.dram_tensor("w1", [KSPLIT, 9, CO], DTYPE, kind="ExternalInput")
    w2_d = nc.dram_tensor("w2", [k2rows, 9, CO], DTYPE, kind="ExternalInput")

    with tile.TileContext(nc) as tc:
        with (
            tc.tile_pool(name="wpool", bufs=1) as wpool,
            tc.tile_pool(name="xpool", bufs=X_BUFS) as xpool,
            tc.tile_pool(name="opool", bufs=O_BUFS) as opool,
            tc.tile_pool(name="pspool", bufs=PS_BUFS, space="PSUM") as pspool,
            tc.For_i(0, repeat, 1) if repeat > 1 else nullcontext(),
        ):
            w1s = wpool.tile([KSPLIT, 9, CO], DTYPE)
            w2s = wpool.tile([k2rows, 9, CO], DTYPE)
            nc.sync.dma_start(out=w1s[:], in_=w1_d[:])
            nc.sync.dma_start(out=w2s[:], in_=w2_d[:])

            nchunk = len(CHUNK_ROWS)
            for t in range(n_tasks):
                kshape = [KSPLIT, FPAD] if p48 else [KSPLIT, H, W]
                k2shape = [k2rows, FPAD] if p48 else [k2rows, H, W]
                k1 = xpool.tile(kshape, DTYPE, tag="k1")
                k2 = xpool.tile(k2shape, DTYPE, tag="k2")
                if not NOLOAD:
                    nc.sync.dma_start(out=k1[:], in_=xs_d[t, 0:KSPLIT])
                    nc.sync.dma_start(out=k2[:], in_=xs_d[t, KSPLIT:KROWS])
                else:
                    nc.vector.memset(k1[:], 0.25)
                    nc.vector.memset(k2[:], 0.25)

                for ci0 in range(0, nchunk, 2):
                    pair = [ci0] + ([ci0 + 1] if ci0 + 1 < nchunk else [])
                    ps_l, o_l = [], []
                    for j, c in enumerate(pair):
                        rows = CHUNK_ROWS[c]
                        # psum tile is one full bank (512 f32) per partition so
                        # the partition-64 slice stays bank-aligned; matmul
                        # writes it as a flat [48, rows*46] AP
                        pw = 48 if p48 else OW
                        ps_full = pspool.tile([112, 512], f32, tag="ps")
                        o_full = opool.tile([112, 10, 48 if p48 else OW], f32, tag="o")
                        p0 = 64 if (COLSPLIT and j == 1) else 0
                        ps_l.append(ps_full[p0 : p0 + CO, 0 : rows * pw])
                        o_l.append(o_full[p0 : p0 + CO, :rows, :])

                    if True:
                        for idx in range(9):
                            ho, wo = divmod(idx, 3)
                            for kt, (ks, ws) in enumerate(((k1, w1s), (k2, w2s))):
                                for j, c in enumerate(pair):
                                    rows = CHUNK_ROWS[c]
                                    oh0 = CHUNK_OH0[c]
                                    if p48:
                                        off = (oh0 + ho) * 48 + wo
                                        rhs = ks[:, off : off + rows * 48]
                                    else:
                                        rhs = ks[:, oh0 + ho : oh0 + ho + rows, wo : wo + OW]
                                    nc.tensor.matmul(
                                        ps_l[j],
                                        lhsT=ws[:, idx, :],
                                        rhs=rhs,
                                        start=(idx == 0 and kt == 0),
                                        stop=(idx == 8 and kt == 1),
                                    )
                    for j, c in enumerate(pair):
                        if NOEPI and not (t == n_tasks - 1 and c == nchunk - 1):
                            continue
                        rows = CHUNK_ROWS[c]
                        if ACTCOPY:
                            nc.scalar.copy(out=o_l[j], in_=ps_l[j])
                        else:
                            nc.vector.tensor_copy(out=o_l[j], in_=ps_l[j])
                        nc.gpsimd.dma_start(
                            out=out_d[t, :, CHUNK_OH0[c] : CHUNK_OH0[c] + rows, :],
                            in_=o_l[j],
                        )
    nc.finalize()
    return nc


def make_in_maps(x, weight, bias3d, n_tasks: int = N_TASKS):
    """Host-side shard + repack into the per-task packed-row layout."""
    npdt = _np_dtype()
    x = np.asarray(x, np.float32)
    weight = np.asarray(weight, np.float32)
    bias3d = np.asarray(bias3d, np.float32)

    # W[(lo*3+do)*24+ci, ho*3+wo, co] = weight[co, ci, lo, do, ho, wo]
    Wr = np.ascontiguousarray(np.transpose(weight, (2, 3, 1, 4, 5, 0))).reshape(
        216, 9, CO
    )
    Wfull = np.zeros((KROWS, 9, CO), np.float32)
    Wfull[:216] = Wr
    Wfull[216, 0, :] = bias3d.sum(axis=0)
    w1 = np.ascontiguousarray(Wfull[:KSPLIT]).astype(npdt)
    w2 = np.ascontiguousarray(Wfull[KSPLIT:]).astype(npdt)

    in_maps = []
    for c in range(8):
        b, lb, db = c // 4, (c // 2) % 2, c % 2
        slab = np.ascontiguousarray(
            x[b, :, 7 * lb : 7 * lb + 9, 7 * db : 7 * db + 9]
        )  # (24, 9, 9, 48, 48)
        s_ci, s_l, s_d, s_h, s_w = slab.strides
        # V[l0, d0, lo, do, ci, h, w] = slab[ci, l0+lo, d0+do, h, w]
        V = np.lib.stride_tricks.as_strided(
            slab,
            shape=(7, 7, 3, 3, CI, H, W),
            strides=(s_l, s_d, s_l, s_d, s_ci, s_h, s_w),
        )
        if PITCH == 48:
            xs = np.zeros((N_TASKS, KROWS, 2306), np.float32)
            xs[:, :216, :2304] = V.reshape(N_TASKS, 216, H * W)
            xs[:, 216] = 1.0
        else:
            xs = np.empty((N_TASKS, KROWS, H, W), np.float32)
            xs[:, :216] = V.reshape(N_TASKS, 216, H, W)
            xs[:, 216] = 1.0
        in_maps.append({"xs": xs[:n_tasks].astype(npdt), "w1": w1, "w2": w2})
    return in_maps


def assemble_output(results):
    out = np.empty((B, CO, OL, OD, OH, OW), np.float32)
    for c in range(8):
        b, lb, db = c // 4, (c // 2) % 2, c % 2
        r = np.asarray(results[c]["out"]).reshape(7, 7, CO, OH, -1)[..., :OW]
        out[b, :, 7 * lb : 7 * lb + 7, 7 * db : 7 * db + 7] = r.transpose(2, 0, 1, 3, 4)
    return out


_NC_CACHE = {}


def _get_program():
    if "nc" not in _NC_CACHE:
        _NC_CACHE["nc"] = build_program()
    return _NC_CACHE["nc"]


def kernel(x, weight, bias3d):
    nc = _get_program()
    in_maps = make_in_maps(x, weight, bias3d)
    res = run_bass_kernel_spmd(nc, in_maps, list(range(8))).results
    return assemble_output(res)
